# revision 13
# baseline (speedup 1.0000x reference)
"""BiLSTM-CRF Trainium2 kernel (8 NeuronCores, SPMD).

Strategy:
  - Core parity 0 runs the forward LSTM direction, parity 1 the backward
    direction (backward = same program fed host-reversed gather indices).
    Core pairs (0,1),(2,3),(4,5),(6,7) are redundant replicas.
  - Per core: on-device embedding gather (fp16 table) -> PE transpose ->
    A = W_ih @ X^T big GEMM (fp16 weights, fp32 psum) kept SBUF-resident ->
    4096-step LSTM recurrence (per-step [2048,512] matvec on PE, fp16
    stationary weights; ACT sigmoid/tanh; DVE cell update) ->
    feats half-projection -> pairwise AllReduce -> Viterbi via max-plus
    block-scan on DVE (exact path via forward+suffix score argmax).
"""
import numpy as np
from contextlib import ExitStack

import concourse.bass as bass
import concourse.tile as tile
from concourse import mybir
from concourse.vector_clock import ScopedClock

P = 128
T = 4096
E = 1024
H = 512
G4 = 2048
V = 50000
TAGSET, START, STOP, NEG_INF = 5, 3, 4, -10000.0
PITCH = T + 1
SS = 32          # viterbi block size (free dim)
NB = T // SS     # 128 viterbi blocks (partition dim)
NEGBIG = -1.0e30

f32 = mybir.dt.float32
f16 = mybir.dt.float16
i32 = mybir.dt.int32

_MAX_DRAIN_WAITS = 1


class TileContextSplit(tile.TileContext):
    """Walrus build in this env allows only 1 sync-wait on the tail Drain;
    split the global-clock waits across extra SP NOPs."""

    def _drain_and_barrier(self, tick_clock, wait_clock):
        nc = self.nc
        drain_inst = nc.sync.drain()
        wait_clock.add_sem_waits(
            drain_inst.ins, ScopedClock({None: tick_clock.global_clock})
        )
        si = drain_inst.ins.sync_info
        if si is not None and len(si.on_wait) > _MAX_DRAIN_WAITS:
            waits = list(si.on_wait)
            drain_inst.ins.sync_info = mybir.SyncInfo(
                on_wait=waits[:_MAX_DRAIN_WAITS], on_update=list(si.on_update)
            )
            rest = waits[_MAX_DRAIN_WAITS:]
            for k in range(0, len(rest), _MAX_DRAIN_WAITS):
                nop = nc.sync.nop()
                nop.ins.sync_info = mybir.SyncInfo(
                    on_wait=rest[k : k + _MAX_DRAIN_WAITS], on_update=[]
                )
        nc.all_engine_barrier()
        assert self.sems is not None
        popped = nc._tile_sem_poison_stack.pop()
        assert popped is self._sem_poison
        nc.clear_and_free_semaphores(list(self.sems.allocated().values()))
        nc.all_engine_barrier()


def _split_multi_waits(nc):
    """This walrus build supports only one sync-wait per instruction; move
    extra waits onto same-engine NoOps inserted just before."""
    n = 0
    for fn in nc.m.functions:
        for bb in fn.blocks:
            insts = list(bb.instructions)
            out = []
            changed = False
            for inst in insts:
                si = getattr(inst, "sync_info", None)
                if si is not None and si.on_wait is not None and len(si.on_wait) > 1:
                    waits = list(si.on_wait)
                    for k, w in enumerate(waits[:-1]):
                        nop = mybir.InstNoOp(
                            name=f"{inst.name}-ws{k}", ins=[], outs=[],
                            engine=inst.engine,
                        )
                        nop.sync_info = mybir.SyncInfo(on_wait=[w], on_update=[])
                        out.append(nop)
                        n += 1
                    inst.sync_info = mybir.SyncInfo(
                        on_wait=[waits[-1]], on_update=list(si.on_update)
                    )
                    changed = True
                out.append(inst)
            if changed:
                bb.instructions = out
    return n


def _ap(base, offs, dims):
    """Build an AP on the same tensor as `base` with free dims `dims`
    (list of [step, count]) at extra element offset `offs` (int or reg)."""
    part = base.ap[0]
    return bass.AP(base.tensor, base.offset + offs, [part] + list(dims))


def mp_product(nc, pool, out_ap, a_ap25, b_ap25, nparts=P):
    """out = A (x) B in max-plus over 5x5 matrices stored flat (r*5+c) in 25
    contiguous elements; batched over `nparts` partitions.
    a_ap25/b_ap25/out_ap are APs whose free dim is the flat 25 block."""
    tmp = pool.tile([P, 125], f32, tag="mp_tmp")
    # tmp[(r,c,m)] = A[(r,m)] + B[(m,c)]
    in0 = _ap(a_ap25, 0, [[5, 5], [0, 5], [1, 5]])
    in1 = _ap(b_ap25, 0, [[0, 5], [1, 5], [5, 5]])
    to = tmp[:nparts, :].rearrange("p (r c m) -> p r c m", r=5, c=5)
    nc.vector.tensor_tensor(out=to, in0=in0, in1=in1, op=mybir.AluOpType.add)
    # out[(r,c)] = max_m tmp
    oo = _ap(out_ap, 0, [[5, 5], [1, 5]])
    nc.vector.tensor_reduce(
        out=oo,
        in_=to,
        axis=mybir.AxisListType.X,
        op=mybir.AluOpType.max,
    )



def _emit_recurrence_chunk(nc, tc, cu, CS, UNROLL, TT, pitch, G4,
                           whh_sb, hbuf, a_chunk, h_stage, c_sb,
                           psg_pool, st_pool):
    f32 = mybir.dt.float32
    f16 = mybir.dt.float16
    with tc.For_i(0, CS, UNROLL) as iv:
        # stage the A slice for this body (one dynamic read)
        a_st = st_pool.tile([P, 16 * UNROLL], f16, tag="a_st")
        nc.gpsimd.tensor_copy(
            a_st[:].rearrange("p (mt u) -> p mt u", mt=16),
            bass.AP(a_chunk[:].tensor, iv + a_chunk[:].offset,
                    [a_chunk[:].ap[0], [CS, 16], [1, UNROLL]]),
        )
        for u in range(UNROLL):
            psg = psg_pool.tile([P, 16], f32, tag="psg")
            for mt in range(16):
                for kc in range(4):
                    rcol = kc * UNROLL + ((u - 1) % UNROLL)
                    nc.tensor.matmul(
                        psg[:, mt : mt + 1],
                        whh_sb[:, kc * G4 + mt * 128 : kc * G4 + mt * 128 + 128],
                        h_stage[:, rcol : rcol + 1],
                        start=(kc == 0),
                        stop=(kc == 3),
                    )
            gates = st_pool.tile([P, 16], f32, tag="gates")
            nc.vector.tensor_tensor(
                out=gates[:],
                in0=psg[:],
                in1=_ap(a_st[:], u, [[UNROLL, 16]]),
                op=mybir.AluOpType.add,
            )
            act = st_pool.tile([P, 16], f32, tag="act")
            nc.scalar.activation(
                act[:, 0:8], gates[:, 0:8], mybir.ActivationFunctionType.Sigmoid
            )
            nc.scalar.activation(
                act[:, 8:12], gates[:, 8:12], mybir.ActivationFunctionType.Tanh
            )
            nc.scalar.activation(
                act[:, 12:16], gates[:, 12:16], mybir.ActivationFunctionType.Sigmoid
            )
            t1 = st_pool.tile([P, 4], f32, tag="t1")
            nc.vector.tensor_mul(t1[:], act[:, 0:4], act[:, 8:12])
            nc.vector.tensor_mul(c_sb[:], act[:, 4:8], c_sb[:])
            nc.vector.tensor_add(c_sb[:], c_sb[:], t1[:])
            tnc = st_pool.tile([P, 4], f32, tag="tnc")
            nc.scalar.activation(tnc[:], c_sb[:], mybir.ActivationFunctionType.Tanh)
            nc.vector.tensor_tensor(
                out=_ap(h_stage[:], u, [[UNROLL, 4]]),
                in0=act[:, 12:16],
                in1=tnc[:],
                op=mybir.AluOpType.mult,
            )
        # record the body's h columns into hbuf (one dynamic write)
        nc.gpsimd.tensor_copy(
            bass.AP(hbuf[:].tensor, iv + (hbuf[:].offset + cu * CS + 1),
                    [hbuf[:].ap[0], [pitch, 4], [1, UNROLL]]),
            h_stage[:].rearrange("p (dc u) -> p dc u", dc=4),
        )


def build_program(num_devices=8, t_total=T, debug=(), split_waits=True):
    """Emit the SPMD program. `debug`: iterable of extra dram dumps among
    {"a", "h", "feats"}."""
    TT = t_total
    TCHUNKS = TT // 512 if TT >= 512 else 0
    assert TT % 128 == 0
    TC = TT // 128
    pitch = TT + 1
    nblocks = TT // SS

    nc = bass.Bass(
        "TRN2", target_bir_lowering=False, debug=False, num_devices=num_devices
    )

    # ---------------- I/O ----------------
    din = {}
    def dt_in(name, shape, dtype):
        din[name] = nc.dram_tensor(name, shape, dtype, kind="ExternalInput").ap()
        return din[name]

    embed_d = dt_in("embed16", [V, E], f16)
    idx_d = dt_in("idx", [P, TC], i32)
    ridx_d = dt_in("ridx", [P, TC], i32)
    wih_d = dt_in("wih", [P, 8 * G4], f16)
    whh_d = dt_in("whh", [P, 4 * G4], f16)
    bias_d = dt_in("bias4h", [P, 16], f32)
    h0c0_d = dt_in("h0c0", [P, 8], f32)
    wout_d = dt_in("wout", [P, 32], f16)
    bout_d = dt_in("bout", [8, 1], f32)
    trb_d = dt_in("trb", [P, 25], f32)
    initv_d = dt_in("initv", [P, 5], f32)
    stopv_d = dt_in("stopv", [P, 5], f32)
    idq_d = dt_in("idq", [P, 25], f32)
    idqc_d = dt_in("idqcol", [P, 1], f32)
    mz_d = dt_in("mz", [P, 25], f32)

    score_o = nc.dram_tensor("score_o", [1, 1], f32, kind="ExternalOutput").ap()
    path_o = nc.dram_tensor("path_o", [nblocks, SS], i32, kind="ExternalOutput").ap()

    dbg = {}
    if "a" in debug:
        dbg["a"] = nc.dram_tensor("dbg_a", [P, 16 * TT], f16, kind="ExternalOutput").ap()
    if "h" in debug:
        dbg["h"] = nc.dram_tensor("dbg_h", [P, 4 * pitch], f16, kind="ExternalOutput").ap()
    if "feats" in debug:
        dbg["feats"] = nc.dram_tensor("dbg_feats", [P, SS * 8], f32, kind="ExternalOutput").ap()

    # collective bounce buffers (pad rows so the t+1-shifted viterbi read stays in bounds)
    fd_d = nc.dram_tensor("fd", [TT + 8, 8], f32).ap()
    fr_d = nc.dram_tensor("fr", [TT + 8, 8], f32).ap()
    # A^T spill: a_dram[r, mt*TT + t] = A[t, mt*128+r]
    a_dram = nc.dram_tensor("a_spill", [P, 16 * TT], f16).ap()

    with TileContextSplit(nc) as tc, ExitStack() as octx:
        # ---- persistent pools ----
        const_pool = octx.enter_context(tc.tile_pool(name="consts", bufs=1))
        big_pool = octx.enter_context(tc.tile_pool(name="big", bufs=1))

        whh_sb = big_pool.tile([P, 4 * G4], f16)
        hbuf = big_pool.tile([P, 4 * pitch], f16)
        bias_sb = const_pool.tile([P, 16], f32)
        h0c0_sb = const_pool.tile([P, 8], f32)
        wout_sb = const_pool.tile([P, 32], f16)
        bout_sb = const_pool.tile([8, 1], f32)
        idx_sb = const_pool.tile([P, TC], i32)
        ridx_sb = const_pool.tile([P, TC], i32)
        trb_sb = const_pool.tile([P, 25], f32)
        initv_sb = const_pool.tile([P, 5], f32)
        stopv_sb = const_pool.tile([P, 5], f32)
        idq_sb = const_pool.tile([P, 25], f32)
        idqc_sb = const_pool.tile([P, 1], f32)
        mz_sb = const_pool.tile([P, 25], f32)
        id16_sb = const_pool.tile([P, P], f16)
        id32_sb = const_pool.tile([P, P], f32)
        c_sb = const_pool.tile([P, 4], f32)

        from concourse.masks import make_identity

        make_identity(nc, id16_sb[:])
        make_identity(nc, id32_sb[:])
        nc.sync.dma_start(whh_sb[:], whh_d[:])
        nc.sync.dma_start(bias_sb[:], bias_d[:])
        nc.sync.dma_start(h0c0_sb[:], h0c0_d[:])
        nc.sync.dma_start(wout_sb[:], wout_d[:])
        nc.sync.dma_start(bout_sb[:], bout_d[:])
        nc.sync.dma_start(idx_sb[:], idx_d[:])
        nc.sync.dma_start(ridx_sb[:], ridx_d[:])
        nc.sync.dma_start(trb_sb[:], trb_d[:])
        nc.sync.dma_start(initv_sb[:], initv_d[:])
        nc.sync.dma_start(stopv_sb[:], stopv_d[:])
        nc.sync.dma_start(idq_sb[:], idq_d[:])
        nc.sync.dma_start(idqc_sb[:], idqc_d[:])
        nc.sync.dma_start(mz_sb[:], mz_d[:])

        # =========== Phase 1: gather -> transpose -> A GEMM ===========
        with ExitStack() as ctx:
            wih_pool = ctx.enter_context(tc.tile_pool(name="wih", bufs=1))
            wih_sb = wih_pool.tile([P, 8 * G4], f16)
            nc.sync.dma_start(wih_sb[:], wih_d[:])

            xg_pool = ctx.enter_context(tc.tile_pool(name="xg", bufs=2))
            xt_pool = ctx.enter_context(tc.tile_pool(name="xt", bufs=2))
            pst_pool = ctx.enter_context(tc.tile_pool(name="pst", bufs=4, space="PSUM"))
            psa_pool = ctx.enter_context(tc.tile_pool(name="psa", bufs=2, space="PSUM"))

            n_ch = TT // 512
            for u in range(n_ch):
                xt_sb = xt_pool.tile([P, 8 * 512], f16, tag="xt")
                for b in range(4):
                    xg = xg_pool.tile([P, E], f16, tag="xg")
                    nc.gpsimd.indirect_dma_start(
                        out=xg[:],
                        out_offset=None,
                        in_=embed_d[:],
                        in_offset=bass.IndirectOffsetOnAxis(
                            ap=idx_sb[:, u * 4 + b : u * 4 + b + 1], axis=0
                        ),
                    )
                    for kc in range(8):
                        pst = pst_pool.tile([P, P], f16, tag="pst")
                        nc.tensor.transpose(
                            pst[:], xg[:, kc * 128 : kc * 128 + 128], id16_sb[:]
                        )
                        nc.vector.tensor_copy(
                            xt_sb[:, kc * 512 + b * 128 : kc * 512 + b * 128 + 128],
                            pst[:],
                        )
                for mt in range(16):
                    psa = psa_pool.tile([P, 512], f32, tag="psa")
                    for kc in range(8):
                        nc.tensor.matmul(
                            psa[:],
                            wih_sb[:, kc * G4 + mt * 128 : kc * G4 + mt * 128 + 128],
                            xt_sb[:, kc * 512 : kc * 512 + 512],
                            start=(kc == 0),
                            stop=(kc == 7),
                        )
                    # copy + bias + cast to fp16 in one DVE op, then spill
                    a_stage = xg_pool.tile([P, 512], f16, tag="a_stage")
                    nc.vector.tensor_tensor(
                        out=a_stage[:],
                        in0=psa[:],
                        in1=_ap(bias_sb[:], mt, [[0, 512]]),
                        op=mybir.AluOpType.add,
                    )
                    nc.sync.dma_start(
                        a_dram[:, mt * TT + u * 512 : mt * TT + u * 512 + 512],
                        a_stage[:],
                    )

        if "a" in dbg:
            with ExitStack() as _dctx:
                _dp = _dctx.enter_context(tc.tile_pool(name="adbg", bufs=2))
                for _mt in range(16):
                    _t = _dp.tile([P, TT], f16, tag="adbg_t")
                    nc.sync.dma_start(_t[:], a_dram[:, _mt * TT : (_mt + 1) * TT])
                    nc.sync.dma_start(dbg["a"][:, _mt * TT : (_mt + 1) * TT], _t[:])

        # =========== Phase 2: recurrence ===========
        # h0 into hbuf cols {dc*pitch} (t=-1 record), c0 into c_sb
        nc.vector.tensor_copy(_ap(hbuf[:], 0, [[pitch, 4]]), h0c0_sb[:, 0:4])
        nc.vector.tensor_copy(c_sb[:], h0c0_sb[:, 4:8])

        UNROLL = 8
        CS = min(1024, TT)
        NCHUNK = TT // CS
        with ExitStack() as ctx:
            psg_pool = ctx.enter_context(tc.tile_pool(name="psg", bufs=4, space="PSUM"))
            st_pool = ctx.enter_context(tc.tile_pool(name="st", bufs=4))
            hst_pool = ctx.enter_context(tc.tile_pool(name="hst", bufs=1))
            ach_pool = ctx.enter_context(tc.tile_pool(name="ach", bufs=2))

            # h_stage col (dc, u) = h_t for t = body_base + u  (fp16)
            # rhs of step u reads col (dc, (u-1) % UNROLL) -- all static APs.
            h_stage = hst_pool.tile([P, 4 * UNROLL], f16)
            # seed: h0 into last column slot (read by first step of first body)
            nc.vector.tensor_copy(
                _ap(h_stage[:], UNROLL - 1, [[UNROLL, 4]]), h0c0_sb[:, 0:4]
            )

            def load_a_chunk(cu):
                ach = ach_pool.tile([P, 16 * CS], f16, tag="ach")
                nc.sync.dma_start(
                    ach[:].rearrange("p (mt t) -> p mt t", mt=16),
                    _ap(a_dram[:], cu * CS, [[TT, 16], [1, CS]]),
                )
                return ach

            a_chunks = {0: load_a_chunk(0)}
            for cu in range(NCHUNK):
                if cu + 1 < NCHUNK:
                    a_chunks[cu + 1] = load_a_chunk(cu + 1)
                a_chunk = a_chunks.pop(cu)
                _emit_recurrence_chunk(
                    nc, tc, cu, CS, UNROLL, TT, pitch, G4,
                    whh_sb, hbuf, a_chunk, h_stage, c_sb,
                    psg_pool, st_pool,
                )

        if "h" in dbg:
            nc.sync.dma_start(dbg["h"][:], hbuf[:])

        # =========== Phase 3: feats half-GEMM + transpose + scatter + allreduce ===========
        with ExitStack() as ctx:
            psf_pool = ctx.enter_context(tc.tile_pool(name="psf", bufs=2, space="PSUM"))
            f_pool = ctx.enter_context(tc.tile_pool(name="fsb", bufs=1))
            ftb_pool = ctx.enter_context(tc.tile_pool(name="ftb", bufs=2))
            psq_pool = ctx.enter_context(tc.tile_pool(name="psq", bufs=2, space="PSUM"))

            f_sb = f_pool.tile([8, TT], f32)
            for n in range(TT // 512):
                psf = psf_pool.tile([8, 512], f32, tag="psf")
                for kc in range(4):
                    nc.tensor.matmul(
                        psf[:],
                        wout_sb[:, kc * 8 : kc * 8 + 8],
                        hbuf[:, kc * pitch + 1 + n * 512 : kc * pitch + 1 + n * 512 + 512],
                        start=(kc == 0),
                        stop=(kc == 3),
                    )
                nc.vector.tensor_tensor(
                    out=f_sb[:, n * 512 : n * 512 + 512],
                    in0=psf[:],
                    in1=_ap(bout_sb[:], 0, [[0, 512]]),
                    op=mybir.AluOpType.add,
                )
            # zero the pad rows of fd (read by the t+1-shifted viterbi load)
            zpad = ftb_pool.tile([8, 8], f32, tag="zpad")
            nc.gpsimd.memset(zpad[:], 0.0)
            nc.sync.dma_start(
                bass.AP(fd_d.tensor, TT * 8, [[8, 8], [1, 8]]), zpad[:]
            )
            # transpose 128-col blocks and scatter rows (with per-core permutation)
            for cblk in range(TC):
                psq = psq_pool.tile([P, 8], f32, tag="psq")
                nc.tensor.transpose(
                    psq[:], f_sb[:, cblk * 128 : cblk * 128 + 128], id32_sb[:8, :8]
                )
                ftb = ftb_pool.tile([P, 8], f32, tag="ftbt")
                nc.vector.tensor_copy(ftb[:], psq[:])
                nc.gpsimd.indirect_dma_start(
                    out=fd_d[:],
                    out_offset=bass.IndirectOffsetOnAxis(
                        ap=ridx_sb[:, cblk : cblk + 1], axis=0
                    ),
                    in_=ftb[:],
                    in_offset=None,
                )
            groups = [[c, c + 1] for c in range(0, num_devices, 2)]
            nc.gpsimd.collective_compute(
                "AllReduce",
                mybir.AluOpType.add,
                replica_groups=groups,
                ins=[fd_d[:].opt()],
                outs=[fr_d[:].opt()],
            )

        # =========== Phase 4: Viterbi ===========
        with ExitStack() as ctx:
            vpool = ctx.enter_context(tc.tile_pool(name="vit", bufs=1))
            vtmp = ctx.enter_context(tc.tile_pool(name="vtmp", bufs=2))
            psv_pool = ctx.enter_context(tc.tile_pool(name="psv", bufs=2, space="PSUM"))

            np_b = nblocks  # partitions used (=128 for T=4096)
            ftb1 = vpool.tile([P, SS * 8], f32)
            ftb2 = vpool.tile([P, SS * 8], f32)
            # ftb1[b, s*8+i] = feats[b*SS+s, i]; ftb2 shifted by one row
            src1 = bass.AP(fr_d.tensor, 0, [[SS * 8, np_b], [8, SS], [1, 8]])
            src2 = bass.AP(fr_d.tensor, 8, [[SS * 8, np_b], [8, SS], [1, 8]])
            nc.sync.dma_start(ftb1[:np_b, :].rearrange("p (s i) -> p s i", s=SS), src1)
            nc.sync.dma_start(ftb2[:np_b, :].rearrange("p (s i) -> p s i", s=SS), src2)
            if "feats" in dbg:
                nc.sync.dma_start(dbg["feats"][:np_b, :], ftb1[:np_b, :])

            M1 = vpool.tile([P, SS * 25], f32)
            M2 = vpool.tile([P, SS * 25], f32)
            for i in range(5):
                # M1[b, s*25 + i*5 + j] = trans[i,j] + feat[b*SS+s, i]
                nc.vector.tensor_tensor(
                    out=_ap(M1[:np_b, :], i * 5, [[25, SS], [1, 5]]),
                    in0=_ap(ftb1[:np_b, :], i, [[8, SS], [0, 5]]),
                    in1=_ap(trb_sb[:np_b, :], i * 5, [[0, SS], [1, 5]]),
                    op=mybir.AluOpType.add,
                )
                # M2[b, s*25 + j*5 + i] = trans[i,j] + feat[b*SS+s+1, i]
                nc.vector.tensor_tensor(
                    out=_ap(M2[:np_b, :], i, [[25, SS], [5, 5]]),
                    in0=_ap(ftb2[:np_b, :], i, [[8, SS], [0, 5]]),
                    in1=_ap(trb_sb[:np_b, :], i * 5, [[0, SS], [1, 5]]),
                    op=mybir.AluOpType.add,
                )
            # fix M2 last element (t = TT-1) to max-plus identity:
            # Z += mz * (idq - Z)
            zslice = M2[:np_b, (SS - 1) * 25 : SS * 25]
            dfix = vtmp.tile([P, 25], f32, tag="dfix")
            nc.vector.tensor_sub(dfix[:np_b, :], idq_sb[:np_b, :], zslice)
            nc.vector.tensor_mul(dfix[:np_b, :], dfix[:np_b, :], mz_sb[:np_b, :])
            nc.vector.tensor_add(zslice, zslice, dfix[:np_b, :])

            # ---- within-block scans ----
            for s in range(1, SS):
                mp_product(
                    nc, vtmp,
                    M1[:np_b, s * 25 : s * 25 + 25],
                    M1[:np_b, s * 25 : s * 25 + 25],
                    M1[:np_b, (s - 1) * 25 : (s - 1) * 25 + 25],
                    nparts=np_b,
                )
            for s in range(SS - 2, -1, -1):
                mp_product(
                    nc, vtmp,
                    M2[:np_b, s * 25 : s * 25 + 25],
                    M2[:np_b, s * 25 : s * 25 + 25],
                    M2[:np_b, (s + 1) * 25 : (s + 1) * 25 + 25],
                    nparts=np_b,
                )

            # ---- level-2 scans over blocks (transpose-shift-transpose rounds) ----
            def level2_scan(prod_src_slice, forward=True):
                """Inclusive scan over block products; returns [P,25] tile of
                EXCLUSIVE prefixes (identity at the boundary block)."""
                cur = vpool.tile([P, 25], f32, tag=f"l2cur_{forward}")
                nc.vector.tensor_copy(cur[:np_b, :], prod_src_slice)
                d = 1
                while d < np_b:
                    pst = psv_pool.tile([25, P], f32, tag="l2ps")
                    nc.tensor.transpose(pst[:25, :np_b], cur[:np_b, :25], id32_sb[:np_b, :np_b])
                    tp = vtmp.tile([25, P], f32, tag="l2tp")
                    nc.vector.tensor_copy(tp[:25, :np_b], pst[:25, :np_b])
                    sh = vtmp.tile([25, P], f32, tag="l2sh")
                    if forward:
                        # sh[:, d:] = tp[:, :-d]; sh[:, :d] = Id
                        nc.vector.tensor_copy(sh[:25, d:np_b], tp[:25, 0 : np_b - d])
                        nc.vector.tensor_copy(
                            sh[:25, 0:d], _ap(idqc_sb[0:25, :], 0, [[0, d]])
                        )
                    else:
                        nc.vector.tensor_copy(sh[:25, 0 : np_b - d], tp[:25, d:np_b])
                        nc.vector.tensor_copy(
                            sh[:25, np_b - d : np_b], _ap(idqc_sb[0:25, :], 0, [[0, d]])
                        )
                    psb = psv_pool.tile([P, 25], f32, tag="l2psb")
                    nc.tensor.transpose(psb[:np_b, :25], sh[:25, :np_b], id32_sb[:25, :25])
                    bsh = vtmp.tile([P, 25], f32, tag="l2bsh")
                    nc.vector.tensor_copy(bsh[:np_b, :], psb[:np_b, :25])
                    mp_product(nc, vtmp, cur[:np_b, :25], cur[:np_b, :25], bsh[:np_b, :25], nparts=np_b)
                    d *= 2
                # exclusive shift by one block
                pst = psv_pool.tile([25, P], f32, tag="l2ps")
                nc.tensor.transpose(pst[:25, :np_b], cur[:np_b, :25], id32_sb[:np_b, :np_b])
                tp = vtmp.tile([25, P], f32, tag="l2tp")
                nc.vector.tensor_copy(tp[:25, :np_b], pst[:25, :np_b])
                sh = vtmp.tile([25, P], f32, tag="l2sh")
                if forward:
                    nc.vector.tensor_copy(sh[:25, 1:np_b], tp[:25, 0 : np_b - 1])
                    nc.vector.tensor_copy(sh[:25, 0:1], _ap(idqc_sb[0:25, :], 0, [[0, 1]]))
                else:
                    nc.vector.tensor_copy(sh[:25, 0 : np_b - 1], tp[:25, 1:np_b])
                    nc.vector.tensor_copy(
                        sh[:25, np_b - 1 : np_b], _ap(idqc_sb[0:25, :], 0, [[0, 1]])
                    )
                psb = psv_pool.tile([P, 25], f32, tag="l2psb")
                nc.tensor.transpose(psb[:np_b, :25], sh[:25, :np_b], id32_sb[:25, :25])
                exc = vpool.tile([P, 25], f32, tag=f"l2exc_{forward}")
                nc.vector.tensor_copy(exc[:np_b, :], psb[:np_b, :25])
                return exc

            pbpx = level2_scan(M1[:np_b, (SS - 1) * 25 : SS * 25], forward=True)
            sbpx = level2_scan(M2[:np_b, 0:25], forward=False)

            # ---- combine: X[b,s] = W[b,s] (x) PBPX[b]; Q[b,s] = Qw[b,s] (x) SBPX[b] ----
            def combine(Wt, Bx, out):
                tmp2 = vtmp.tile([P, SS * 25], f32, tag="cmb")
                first = True
                for m in range(5):
                    tgt = out if first else tmp2
                    nc.vector.tensor_tensor(
                        out=_ap(tgt[:np_b, :], 0, [[25, SS], [5, 5], [1, 5]]),
                        in0=_ap(Wt[:np_b, :], m, [[25, SS], [5, 5], [0, 5]]),
                        in1=_ap(Bx[:np_b, :], m * 5, [[0, SS], [0, 5], [1, 5]]),
                        op=mybir.AluOpType.add,
                    )
                    if not first:
                        nc.vector.tensor_max(out[:np_b, :], out[:np_b, :], tmp2[:np_b, :])
                    first = False

            Xf = vpool.tile([P, SS * 25], f32)
            Qf = vpool.tile([P, SS * 25], f32)
            combine(M1, pbpx, Xf)
            combine(M2, sbpx, Qf)

            # ---- FV / SV vectors ----
            def apply_vec(Xt, vec, out):
                # out[b, s*5 + r] = max_c Xt[b, s*25 + r*5 + c] + vec[c]
                tmp2 = vtmp.tile([P, SS * 5], f32, tag="apv")
                first = True
                for c in range(5):
                    tgt = out if first else tmp2
                    nc.vector.tensor_tensor(
                        out=_ap(tgt[:np_b, :], 0, [[5, SS], [1, 5]]),
                        in0=_ap(Xt[:np_b, :], c, [[25, SS], [5, 5]]),
                        in1=_ap(vec[:np_b, :], c, [[0, SS], [0, 5]]),
                        op=mybir.AluOpType.add,
                    )
                    if not first:
                        nc.vector.tensor_max(
                            out[:np_b, 0 : SS * 5], out[:np_b, 0 : SS * 5], tmp2[:np_b, 0 : SS * 5]
                        )
                    first = False

            FV = vpool.tile([P, SS * 5], f32)
            SV = vpool.tile([P, SS * 5], f32)
            apply_vec(Xf, initv_sb, FV)
            apply_vec(Qf, stopv_sb, SV)
            tot = vpool.tile([P, SS * 5], f32)
            nc.vector.tensor_add(tot[:np_b, :], FV[:np_b, :], SV[:np_b, :])

            # ---- path = argmax_r tot (first-max tie break) ----
            mx = vpool.tile([P, SS], f32)
            nc.vector.tensor_copy(mx[:np_b, :], _ap(tot[:np_b, :], 0, [[5, SS]]))
            for r in range(1, 5):
                nc.vector.tensor_max(
                    mx[:np_b, :], mx[:np_b, :], _ap(tot[:np_b, :], r, [[5, SS]])
                )
            pathf = vpool.tile([P, SS], f32)
            nc.gpsimd.memset(pathf[:], 4.0)
            eq = vtmp.tile([P, SS], f32, tag="eq")
            dsel = vtmp.tile([P, SS], f32, tag="dsel")
            for r in range(3, -1, -1):
                nc.vector.tensor_tensor(
                    out=eq[:np_b, :],
                    in0=_ap(tot[:np_b, :], r, [[5, SS]]),
                    in1=mx[:np_b, :],
                    op=mybir.AluOpType.is_equal,
                )
                # path = path + eq*(r - path)
                nc.vector.tensor_scalar(
                    out=dsel[:np_b, :],
                    in0=pathf[:np_b, :],
                    scalar1=-1.0,
                    scalar2=float(r),
                    op0=mybir.AluOpType.mult,
                    op1=mybir.AluOpType.add,
                )
                nc.vector.tensor_mul(dsel[:np_b, :], dsel[:np_b, :], eq[:np_b, :])
                nc.vector.tensor_add(pathf[:np_b, :], pathf[:np_b, :], dsel[:np_b, :])
            pathi = vpool.tile([P, SS], i32)
            nc.vector.tensor_copy(pathi[:np_b, :], pathf[:np_b, :])
            nc.sync.dma_start(path_o[:], pathi[:np_b, :])

            # ---- score = max_r tot at t = TT-1 (partition np_b-1, s = SS-1) ----
            sc = vpool.tile([P, 1], f32)
            base = max(0, np_b - 32)
            nc.vector.tensor_reduce(
                out=sc[base:np_b, 0:1],
                in_=tot[base:np_b, (SS - 1) * 5 : SS * 5],
                axis=mybir.AxisListType.X,
                op=mybir.AluOpType.max,
            )
            nc.sync.dma_start(score_o[:], sc[np_b - 1 : np_b, 0:1])

    if split_waits:
        _split_multi_waits(nc)
    return nc


# ======================= host side =======================

def prep_core_inputs(inputs, parity, t_total=T):
    TT = t_total
    sent = np.asarray(inputs["sentence"]).astype(np.int64)[:t_total]
    seq = sent if parity == 0 else sent[::-1].copy()
    embed16 = prep_core_inputs._embed_cache
    if parity == 0:
        wih, whh, b = inputs["W_ih_f"], inputs["W_hh_f"], inputs["b_f"]
    else:
        wih, whh, b = inputs["W_ih_b"], inputs["W_hh_b"], inputs["b_b"]
    wih = np.asarray(wih, np.float32)
    whh = np.asarray(whh, np.float32)
    b = np.asarray(b, np.float32)
    h0 = np.asarray(inputs["h0"], np.float32)[parity]
    c0 = np.asarray(inputs["c0"], np.float32)[parity]
    wout_full = np.asarray(inputs["W_out"], np.float32)
    bout = np.asarray(inputs["b_out"], np.float32)
    trans = np.asarray(inputs["transitions"], np.float32)

    d = {}
    d["embed16"] = embed16
    d["idx"] = np.ascontiguousarray(seq.reshape(TT // 128, 128).T.astype(np.int32))
    perm = np.arange(TT, dtype=np.int32) if parity == 0 else np.arange(TT - 1, -1, -1, dtype=np.int32)
    d["ridx"] = np.ascontiguousarray(perm.reshape(TT // 128, 128).T)
    d["wih"] = np.ascontiguousarray(
        wih.T.reshape(8, 128, G4).transpose(1, 0, 2).reshape(128, 8 * G4)
    ).astype(np.float16)
    d["whh"] = np.ascontiguousarray(
        whh.T.reshape(4, 128, G4).transpose(1, 0, 2).reshape(128, 4 * G4)
    ).astype(np.float16)
    d["bias4h"] = np.ascontiguousarray(b.reshape(16, 128).T)
    d["h0c0"] = np.ascontiguousarray(
        np.concatenate([h0.reshape(4, 128).T, c0.reshape(4, 128).T], axis=1)
    )
    wh = wout_full[:, parity * 512 : (parity + 1) * 512]  # [5, 512]
    warr = np.zeros((128, 4, 8), np.float32)
    warr[:, :, :5] = wh.T.reshape(4, 128, 5).transpose(1, 0, 2)
    d["wout"] = np.ascontiguousarray(warr.reshape(128, 32)).astype(np.float16)
    bo = np.zeros((8, 1), np.float32)
    if parity == 0:
        bo[:5, 0] = bout
    d["bout"] = bo
    d["trb"] = np.tile(trans.reshape(1, 25), (128, 1)).astype(np.float32)
    init = np.full(5, NEG_INF, np.float32)
    init[START] = 0.0
    d["initv"] = np.tile(init.reshape(1, 5), (128, 1))
    d["stopv"] = np.tile(trans[STOP].reshape(1, 5), (128, 1))
    idq = np.where(np.eye(5, dtype=bool), 0.0, NEGBIG).astype(np.float32)
    d["idq"] = np.tile(idq.reshape(1, 25), (128, 1))
    idqcol = np.full((128, 1), NEGBIG, np.float32)
    for e in range(25):
        idqcol[e, 0] = 0.0 if (e // 5 == e % 5) else NEGBIG
    d["idqcol"] = idqcol
    mz = np.zeros((128, 25), np.float32)
    mz[t_total // SS - 1, :] = 1.0
    d["mz"] = mz
    return d


def kernel(**inputs):
    from concourse.bass_utils import run_bass_kernel_spmd

    prep_core_inputs._embed_cache = np.asarray(inputs["embed"], np.float32).astype(
        np.float16
    )
    nc = build_program(num_devices=8)
    in_maps = [prep_core_inputs(inputs, c % 2) for c in range(8)]
    res = run_bass_kernel_spmd(nc, in_maps, core_ids=list(range(8)))
    score = np.float32(res.results[0]["score_o"][0, 0])
    path = res.results[0]["path_o"].reshape(T).astype(np.int32)
    return score, path
